# revision 1
# baseline (speedup 1.0000x reference)
"""DETR scene-graph predicate head on 8 Trainium2 NeuronCores.

Math: logits[l,b,r,:] = concat(hs[l,b,q_sub], hs[l,b,q_obj]) @ W_pred.T + b_pred
where q_sub/q_obj are derived from (tgt_perm inverse, relationships,
src_indices) — pure integer index math, done on host.

Strategy (batch axis sharded 8 ways; L*B/8 = 192 (layer,image) blocks/core):
  - Host builds, per block, a [101, 384] bf16 row: hs_block [101, 256] next to
    a one-hot selector [101, 128] (col j selects query q_sub[j], j<64, or
    q_obj[j-64]). Blocks are packed in groups of G=8 into one padded
    [128, G*384] DMA (128 partitions keeps all 16 SDMA engines engaged;
    SWDGE/gpsimd queue — the HWDGE path runs at single-engine rate here).
  - Phase A (gather+transpose fused): pAB = hs_chunk.T @ onehot gives the
    gathered pair representation already d-on-partitions, two matmuls per
    block into one shared psum bank per block-pair, one DVE/ACT cast copy
    to bf16 (alternating engines).
  - Phase B: logits[r, p] accumulates 4 matmuls (2 d-chunks x sub/obj)
    against W_pred.T chunks; blocks 2k/2k+1 run concurrently in the left/
    right PE array halves via tile_position (0,0)/(0,64), outputs stacked on
    psum partitions 0:64/64:128 of one group-wide bank.
  - One bias add (DVE) + one store (scalar-queue DMA) per group; host
    unpacks the [NG, 128, 4*51] layout back to [L, B, R, P].
  - A ~4.5us dense-matmul preamble warms the PE clock (HAM) to 2.4 GHz.

hs and W_pred are bf16 on-chip (one-hot gather is exact in bf16; psum
accumulates f32), giving ~2.4e-3 relative error vs the f32 reference.
"""

import sys

import numpy as np

L, B, Q1, D = 6, 256, 101, 256
M, R, P = 64, 64, 51
NCORES = 8
BLOC = B // NCORES          # images per core
NB = L * BLOC               # (layer, image) blocks per core
PK = D + 2 * R              # packed row width: 256 hs + 128 onehot
G = 8                       # blocks per DMA group
NG = NB // G                # groups per core

_CACHE = {}


def _build_program():
    import concourse.bacc as bacc
    import concourse.mybir as mybir
    import concourse.tile as tile
    from contextlib import ExitStack

    f32 = mybir.dt.float32
    bf16 = mybir.dt.bfloat16
    nc = bacc.Bacc("TRN2", target_bir_lowering=False, debug=False)

    # phase-B col-packing: blocks 2k / 2k+1 share the PE array via
    # tile_position (0,0)/(0,64); outputs land on psum partitions 0:64 /
    # 64:128 at column slot k -> group output is [128, (G//2)*P].
    GH = G // 2
    pk = nc.dram_tensor("pk", [NG, 128, G * PK], bf16, kind="ExternalInput").ap()
    wt = nc.dram_tensor("wt", [128, 4 * P], bf16, kind="ExternalInput").ap()
    bias = nc.dram_tensor("bias", [128, GH * P], f32, kind="ExternalInput").ap()
    out = nc.dram_tensor("out", [NG, 128, GH * P], f32, kind="ExternalOutput").ap()

    with tile.TileContext(nc) as tc, ExitStack() as ctx:
        const = ctx.enter_context(tc.tile_pool(name="const", bufs=1))
        inp = ctx.enter_context(tc.tile_pool(name="inp", bufs=5))
        rep = ctx.enter_context(tc.tile_pool(name="rep", bufs=6))
        outp = ctx.enter_context(tc.tile_pool(name="outp", bufs=3))
        psA = ctx.enter_context(tc.tile_pool(name="psA", bufs=4, space="PSUM"))
        psO = ctx.enter_context(tc.tile_pool(name="psO", bufs=2, space="PSUM"))

        wt_t = const.tile([128, 4 * P], bf16)
        nc.sync.dma_start(out=wt_t[:], in_=wt[:])
        bias_t = const.tile([128, GH * P], f32)
        nc.sync.dma_start(out=bias_t[:], in_=bias[:])

        # HAM warm-up: dense N=512 matmuls push the PE clock 1.2 -> 2.4 GHz
        wu = const.tile([128, 512], bf16)
        nc.vector.memset(wu[:], 0.0)
        wps = psA.tile([128, 512], f32, tag="pAB")
        for _ in range(20):
            nc.tensor.matmul(out=wps[:], lhsT=wu[:, 0:128], rhs=wu[:],
                             start=True, stop=True)


        for g in range(NG):
            # one contiguous load per group of G blocks (bf16)
            pk_t = inp.tile([128, G * PK], bf16, tag="pk")
            nc.gpsimd.dma_start(out=pk_t[:], in_=pk[g])
            o_t = outp.tile([128, GH * P], f32, tag="o")
            # all G blocks' phase-B outputs share one psum bank tile
            pO = psO.tile([128, GH * P], f32, tag="pO")

            # pairs of blocks (2k, 2k+1) flow together: 4 gather matmuls into
            # one full psum bank, one cast copy, then 8 col-packed predicate
            # matmuls (left/right array halves run concurrently).
            for k in range(GH):
                j0, j1 = 2 * k, 2 * k + 1
                pAB = psA.tile([128, 512], f32, tag="pAB")
                for s, j in enumerate((j0, j1)):
                    hs_t = pk_t[0:Q1, j * PK:j * PK + D]
                    oh_t = pk_t[0:Q1, j * PK + D:(j + 1) * PK]
                    # pAB cols [s*256 : s*256+256]: [d-chunk0 | d-chunk1],
                    # each [sub 64 | obj 64]
                    nc.tensor.matmul(out=pAB[:, s * 256:s * 256 + 2 * R],
                                     lhsT=hs_t[:, 0:128], rhs=oh_t[:],
                                     start=True, stop=True)
                    nc.tensor.matmul(out=pAB[:, s * 256 + 2 * R:s * 256 + 4 * R],
                                     lhsT=hs_t[:, 128:256], rhs=oh_t[:],
                                     start=True, stop=True)
                bAB = rep.tile([128, 512], bf16, tag="bAB")
                if k % 2 == 0:
                    nc.vector.tensor_copy(out=bAB[:], in_=pAB[:])
                else:
                    nc.scalar.copy(out=bAB[:], in_=pAB[:])

                o0 = pO[0:R, k * P:(k + 1) * P]
                o1 = pO[R:2 * R, k * P:(k + 1) * P]
                for c, (lo, hi) in enumerate(
                        [(0, R), (2 * R, 3 * R), (R, 2 * R), (3 * R, 4 * R)]):
                    wch = wt_t[:, c * P:(c + 1) * P]
                    nc.tensor.matmul(out=o0, lhsT=bAB[:, lo:hi], rhs=wch,
                                     start=(c == 0), stop=(c == 3),
                                     tile_position=(0, 0))
                    nc.tensor.matmul(out=o1, lhsT=bAB[:, 256 + lo:256 + hi],
                                     rhs=wch,
                                     start=(c == 0), stop=(c == 3),
                                     tile_position=(0, 64))

            # one bias add for the whole group, one store per group
            nc.vector.tensor_add(out=o_t[:], in0=pO[:], in1=bias_t[:])
            nc.scalar.dma_start(out=out[g], in_=o_t[:])

    nc.compile()
    return nc


def _host_indices(src_indices, tgt_perm, relationships):
    """q_sub, q_obj: [L, B, R] int64 — matched query slot per relation."""
    src = np.asarray(src_indices, dtype=np.int64)
    tgt = np.asarray(tgt_perm, dtype=np.int64)
    rel = np.asarray(relationships, dtype=np.int64)

    # lookup[l, b, tgt[l, b, k]] = k
    lookup = np.empty((L, B, M), dtype=np.int64)
    li = np.arange(L)[:, None, None]
    bi = np.arange(B)[None, :, None]
    lookup[li, bi, tgt] = np.broadcast_to(np.arange(M), (L, B, M))

    sub_t = np.broadcast_to(rel[None, :, :, 0], (L, B, R))
    obj_t = np.broadcast_to(rel[None, :, :, 1], (L, B, R))
    pos_sub = np.take_along_axis(lookup, sub_t, axis=2)
    pos_obj = np.take_along_axis(lookup, obj_t, axis=2)
    q_sub = np.take_along_axis(src, pos_sub, axis=2)
    q_obj = np.take_along_axis(src, pos_obj, axis=2)
    return q_sub, q_obj


def _host_prepare(hs, src_indices, tgt_perm, relationships, W_pred, b_pred):
    """Build per-core input maps."""
    hs = np.asarray(hs, dtype=np.float32)
    W = np.asarray(W_pred, dtype=np.float32)
    b = np.asarray(b_pred, dtype=np.float32)

    q_sub, q_obj = _host_indices(src_indices, tgt_perm, relationships)
    q_cat = np.concatenate([q_sub, q_obj], axis=-1)          # [L, B, 2R]
    onehot = (np.arange(Q1)[None, None, :, None] == q_cat[:, :, None, :])
    onehot = onehot.astype(np.float32)                        # [L, B, Q1, 2R]

    import ml_dtypes
    bf16 = ml_dtypes.bfloat16

    packed = np.zeros((L, B, 128, PK), dtype=bf16)
    packed[:, :, :Q1, :D] = hs.astype(bf16)
    packed[:, :, :Q1, D:] = onehot

    # W_pred [P, 2D] -> Wt [2D, P] -> packed [128, 4*P] chunk-major
    wt_packed = np.ascontiguousarray(
        W.T.reshape(4, 128, P).transpose(1, 0, 2).reshape(128, 4 * P)
    ).astype(bf16)
    bias_b = np.ascontiguousarray(np.tile(b[None, :], (128, G // 2)))  # [128, GH*P]

    in_maps = []
    for c in range(NCORES):
        sl = slice(c * BLOC, (c + 1) * BLOC)
        pk_core = packed[:, sl].reshape(NB, 128, PK)
        # group-major layout: [NG, Q1, G*PK], block j of group at cols j*PK
        pk_core = np.ascontiguousarray(
            pk_core.reshape(NG, G, 128, PK).transpose(0, 2, 1, 3)
            .reshape(NG, 128, G * PK))
        in_maps.append({
            "pk": pk_core,
            "wt": wt_packed,
            "bias": bias_b,
        })
    return in_maps


def kernel(hs, src_indices, tgt_perm, relationships, W_pred, b_pred):
    if "concourse" not in sys.modules:
        try:
            import concourse  # noqa: F401
        except ImportError:
            sys.path.insert(0, "/opt/trn_rl_repo")
    from concourse import bass_utils

    in_maps = _host_prepare(hs, src_indices, tgt_perm, relationships,
                            W_pred, b_pred)
    if "nc" not in _CACHE:
        _CACHE["nc"] = _build_program()
    nc = _CACHE["nc"]

    res = bass_utils.run_bass_kernel_spmd(nc, in_maps, list(range(NCORES)))
    outs = []
    for c in range(NCORES):
        o = res.results[c]["out"]                      # [NG, 128, GH*P]
        o = o.reshape(NG, 2, R, G // 2, P).transpose(0, 3, 1, 2, 4)
        outs.append(o.reshape(L, BLOC, R, P))
    return np.concatenate(outs, axis=1)



# revision 3
# speedup vs baseline: 1.3591x; 1.3591x over previous
"""DETR scene-graph predicate head on 8 Trainium2 NeuronCores.

Math: logits[l,b,r,:] = concat(hs[l,b,q_sub], hs[l,b,q_obj]) @ W_pred.T + b_pred
where q_sub/q_obj are derived from (tgt_perm inverse, relationships,
src_indices) — pure integer index math, done on host.

Strategy (batch axis sharded 8 ways; L*B/8 = 192 (layer,image) blocks/core):
  - The host performs the gather itself (it owns the indices anyway) and
    ships the gathered pair representations pre-transposed: for each block,
    four [128, 512] bf16 chunks c=(sub/obj, d-half) with d on partitions and
    (block j, relation r) on columns. This removes the on-chip one-hot
    gather matmuls, the psum->bf16 cast traffic, and halves input DMA bytes
    vs shipping hs + one-hot selectors.
  - Kernel per group of G=8 blocks: 4 accumulating matmuls
    (lhsT = W chunk [128, 51] stationary, rhs streams 512 cols = 8 blocks
    x 64 relations) into one [51, 512] psum bank, then one per-partition
    bias add (DVE/ACT alternating) casting to bf16, and one store.
  - Input DMA: one [128, 8KB/partition] gpsimd-queue (SWDGE) load per
    super-group of 2 groups — same transfer geometry that measured
    ~400 GB/s aggregate in the previous kernel.
  - A dense-matmul preamble warms the PE clock (HAM) 1.2 -> 2.4 GHz.

hs and W_pred are bf16 on-chip (gather is exact; psum accumulates f32) and
the output travels bf16 over DMA before the host casts to f32, giving
~3e-3 relative error vs the f32 reference.
"""

import sys

import numpy as np

L, B, Q1, D = 6, 256, 101, 256
M, R, P = 64, 64, 51
NCORES = 8
BLOC = B // NCORES          # images per core
NB = L * BLOC               # (layer, image) blocks per core
G = 8                       # blocks per group (one psum bank of 8*64 cols)
NG = NB // G                # groups per core
GD = 2                      # groups per DMA super-group
NG2 = NG // GD

_CACHE = {}


def _build_program():
    import concourse.bacc as bacc
    import concourse.mybir as mybir
    import concourse.tile as tile
    from contextlib import ExitStack

    f32 = mybir.dt.float32
    bf16 = mybir.dt.bfloat16
    nc = bacc.Bacc("TRN2", target_bir_lowering=False, debug=False)

    pg = nc.dram_tensor("pg", [NG2, 128, GD * 4 * 512], bf16,
                        kind="ExternalInput").ap()
    wt = nc.dram_tensor("wt", [128, 4 * P], bf16, kind="ExternalInput").ap()
    bias = nc.dram_tensor("bias", [P, 1], f32, kind="ExternalInput").ap()
    out = nc.dram_tensor("out", [NG2, P, GD * 512], bf16,
                         kind="ExternalOutput").ap()

    with tile.TileContext(nc) as tc, ExitStack() as ctx:
        const = ctx.enter_context(tc.tile_pool(name="const", bufs=1))
        inp = ctx.enter_context(tc.tile_pool(name="inp", bufs=3))
        outp = ctx.enter_context(tc.tile_pool(name="outp", bufs=3))
        psW = ctx.enter_context(tc.tile_pool(name="psW", bufs=1, space="PSUM"))
        psO = ctx.enter_context(tc.tile_pool(name="psO", bufs=4, space="PSUM"))

        wt_t = const.tile([128, 4 * P], bf16)
        nc.sync.dma_start(out=wt_t[:], in_=wt[:])
        bias_t = const.tile([P, 1], f32)
        nc.sync.dma_start(out=bias_t[:], in_=bias[:])

        # HAM warm-up: dense N=512 matmuls push the PE clock 1.2 -> 2.4 GHz
        wu = const.tile([128, 512], bf16)
        nc.vector.memset(wu[:], 0.0)
        wps = psW.tile([128, 512], f32, tag="wu")
        for _ in range(20):
            nc.tensor.matmul(out=wps[:], lhsT=wu[:, 0:128], rhs=wu[:],
                             start=True, stop=True)

        for g2 in range(NG2):
            pg_t = inp.tile([128, GD * 4 * 512], bf16, tag="pg")
            nc.gpsimd.dma_start(out=pg_t[:], in_=pg[g2])
            o_t = outp.tile([P, GD * 512], bf16, tag="o")
            for gs in range(GD):
                pO = psO.tile([P, 512], f32, tag="pO")
                for c in range(4):
                    cc = gs * 4 + c
                    nc.tensor.matmul(out=pO[:],
                                     lhsT=wt_t[:, c * P:(c + 1) * P],
                                     rhs=pg_t[:, cc * 512:(cc + 1) * 512],
                                     start=(c == 0), stop=(c == 3))
                osl = o_t[:, gs * 512:(gs + 1) * 512]
                if gs % 2 == 0:
                    nc.vector.tensor_scalar_add(out=osl, in0=pO[:],
                                                scalar1=bias_t[:])
                else:
                    nc.scalar.add(out=osl, in_=pO[:], add=bias_t[:])
            nc.scalar.dma_start(out=out[g2], in_=o_t[:])

    nc.compile()
    return nc


def _host_indices(src_indices, tgt_perm, relationships):
    """q_sub, q_obj: [L, B, R] int64 — matched query slot per relation."""
    src = np.asarray(src_indices, dtype=np.int64)
    tgt = np.asarray(tgt_perm, dtype=np.int64)
    rel = np.asarray(relationships, dtype=np.int64)

    # lookup[l, b, tgt[l, b, k]] = k
    lookup = np.empty((L, B, M), dtype=np.int64)
    li = np.arange(L)[:, None, None]
    bi = np.arange(B)[None, :, None]
    lookup[li, bi, tgt] = np.broadcast_to(np.arange(M), (L, B, M))

    sub_t = np.broadcast_to(rel[None, :, :, 0], (L, B, R))
    obj_t = np.broadcast_to(rel[None, :, :, 1], (L, B, R))
    pos_sub = np.take_along_axis(lookup, sub_t, axis=2)
    pos_obj = np.take_along_axis(lookup, obj_t, axis=2)
    q_sub = np.take_along_axis(src, pos_sub, axis=2)
    q_obj = np.take_along_axis(src, pos_obj, axis=2)
    return q_sub, q_obj


def _host_prepare(hs, src_indices, tgt_perm, relationships, W_pred, b_pred):
    """Build per-core input maps."""
    import ml_dtypes
    bf16 = ml_dtypes.bfloat16

    hs_bf = np.asarray(hs, dtype=np.float32).astype(bf16)
    W = np.asarray(W_pred, dtype=np.float32)
    b = np.asarray(b_pred, dtype=np.float32)

    q_sub, q_obj = _host_indices(src_indices, tgt_perm, relationships)
    q_cat = np.concatenate([q_sub, q_obj], axis=-1)          # [L, B, 2R]
    # gathered[l, b, j, :] = hs[l, b, q_cat[l, b, j], :]
    gathered = np.take_along_axis(hs_bf, q_cat[..., None], axis=2)

    # W chunks: wt[:, c*P + p] = W[p, c*128 + dd]
    wt_packed = np.ascontiguousarray(
        W.reshape(P, 4, 128).transpose(2, 1, 0).reshape(128, 4 * P)
    ).astype(bf16)
    bias_col = np.ascontiguousarray(b[:, None])               # [P, 1]

    in_maps = []
    for core in range(NCORES):
        sl = slice(core * BLOC, (core + 1) * BLOC)
        # cols per super-group must be (gs, c=(so, dh), j, r), d on partitions
        arr = gathered[:, sl].reshape(NB, 2, R, 2, 128)   # [nb, so, r, dh, dd]
        arr = arr.reshape(NG2, GD, G, 2, R, 2, 128)       # [g2,gs,j,so,r,dh,dd]
        pg_core = np.ascontiguousarray(
            arr.transpose(0, 6, 1, 3, 5, 2, 4))           # [g2,dd,gs,so,dh,j,r]
        in_maps.append({
            "pg": pg_core.reshape(NG2, 128, GD * 4 * 512),
            "wt": wt_packed,
            "bias": bias_col,
        })
    return in_maps


def kernel(hs, src_indices, tgt_perm, relationships, W_pred, b_pred):
    if "concourse" not in sys.modules:
        try:
            import concourse  # noqa: F401
        except ImportError:
            sys.path.insert(0, "/opt/trn_rl_repo")
    from concourse import bass_utils

    in_maps = _host_prepare(hs, src_indices, tgt_perm, relationships,
                            W_pred, b_pred)
    if "nc" not in _CACHE:
        _CACHE["nc"] = _build_program()
    nc = _CACHE["nc"]

    res = bass_utils.run_bass_kernel_spmd(nc, in_maps, list(range(NCORES)))
    outs = []
    for core in range(NCORES):
        o = res.results[core]["out"]                  # [NG2, P, GD*512] bf16
        o = np.asarray(o, dtype=np.float32)
        o = o.reshape(NG2, P, GD * G, R).transpose(0, 2, 3, 1)
        outs.append(o.reshape(L, BLOC, R, P))
    return np.concatenate(outs, axis=1)


# revision 5
# speedup vs baseline: 1.6159x; 1.1890x over previous
"""DETR scene-graph predicate head on 8 Trainium2 NeuronCores.

Math: logits[l,b,r,:] = concat(hs[l,b,q_sub], hs[l,b,q_obj]) @ W_pred.T + b_pred
where q_sub/q_obj are derived from (tgt_perm inverse, relationships,
src_indices) — pure integer index math, done on host.

Strategy (batch axis sharded 8 ways; L*B/8 = 192 (layer,image) blocks/core):
  - The host performs the gather itself (it owns the indices anyway) and
    ships the gathered pair representations pre-transposed: per block, four
    [128, 64] bf16 chunks c=(sub/obj, d-half) with d on partitions and
    relation r on columns. This removes the on-chip one-hot gather matmuls,
    the psum->bf16 cast traffic, and halves input DMA bytes vs shipping
    hs + one-hot selectors.
  - Kernel per group of G=8 blocks: 4 accumulating matmuls (lhsT = W chunk
    [128, 51] stationary, rhs streams 512 cols = 8 blocks x 64 relations)
    into a [51, 512] psum region. Groups are paired via tile_position
    (0,0)/(0,64): outputs land on psum partitions 0:51 / 64:115 of one
    bank, so stores are 128-partition (fans across all DMA engines instead
    of 3) at the price of 25% dead rows.
  - Input DMA: one [128, 16KB/partition] gpsimd-queue (SWDGE) load per
    super-group of GD=4 groups — long descriptors, few loads.
  - Dummy matmuls at each super-group top keep the PE clock (HAM) at
    2.4 GHz through DMA-paced stretches.

hs and W_pred are bf16 on-chip (gather is exact; psum accumulates f32) and
the output travels bf16 over DMA before the host casts to f32, giving
~3e-3 relative error vs the f32 reference.
"""

import sys

import numpy as np

L, B, Q1, D = 6, 256, 101, 256
M, R, P = 64, 64, 51
NCORES = 8
BLOC = B // NCORES          # images per core
NB = L * BLOC               # (layer, image) blocks per core
G = 8                       # blocks per group (one 512-col psum region)
NG = NB // G                # groups per core
GD = 4                      # groups per DMA super-group
NSG = NG // GD              # super-groups per core

_CACHE = {}


def _build_program():
    import concourse.bacc as bacc
    import concourse.mybir as mybir
    import concourse.tile as tile
    from contextlib import ExitStack

    f32 = mybir.dt.float32
    bf16 = mybir.dt.bfloat16
    nc = bacc.Bacc("TRN2", target_bir_lowering=False, debug=False)

    pg = nc.dram_tensor("pg", [NSG, 128, GD * 4 * 512], bf16,
                        kind="ExternalInput").ap()
    wt = nc.dram_tensor("wt", [128, 4 * P], bf16, kind="ExternalInput").ap()
    bias = nc.dram_tensor("bias", [128, 1], f32, kind="ExternalInput").ap()
    out = nc.dram_tensor("out", [NSG, 128, (GD // 2) * 512], bf16,
                         kind="ExternalOutput").ap()

    with tile.TileContext(nc) as tc, ExitStack() as ctx:
        const = ctx.enter_context(tc.tile_pool(name="const", bufs=1))
        inp = ctx.enter_context(tc.tile_pool(name="inp", bufs=3))
        outp = ctx.enter_context(tc.tile_pool(name="outp", bufs=3))
        psW = ctx.enter_context(tc.tile_pool(name="psW", bufs=1, space="PSUM"))
        psO = ctx.enter_context(tc.tile_pool(name="psO", bufs=4, space="PSUM"))

        wt_t = const.tile([128, 4 * P], bf16)
        nc.sync.dma_start(out=wt_t[:], in_=wt[:])
        bias_t = const.tile([128, 1], f32)
        nc.sync.dma_start(out=bias_t[:], in_=bias[:])

        # HAM warm-up: dense N=512 matmuls push the PE clock 1.2 -> 2.4 GHz
        wu = const.tile([128, 512], bf16)
        nc.vector.memset(wu[:], 0.0)
        wps = psW.tile([128, 512], f32, tag="wu")
        for _ in range(20):
            nc.tensor.matmul(out=wps[:], lhsT=wu[:, 0:128], rhs=wu[:],
                             start=True, stop=True)

        for sg in range(NSG):
            pg_t = inp.tile([128, GD * 4 * 512], bf16, tag="pg")
            nc.gpsimd.dma_start(out=pg_t[:], in_=pg[sg])
            o_t = outp.tile([128, (GD // 2) * 512], bf16, tag="o")
            # keep-warm matmuls run while the PE waits on this load
            for _ in range(2):
                nc.tensor.matmul(out=wps[:], lhsT=wu[:, 0:128], rhs=wu[:],
                                 start=True, stop=True)
            for pp in range(GD // 2):
                pO = psO.tile([128, 512], f32, tag="pO")
                for e in range(2):
                    gs = 2 * pp + e
                    rows = slice(64 * e, 64 * e + P)
                    for c in range(4):
                        cc = gs * 4 + c
                        nc.tensor.matmul(out=pO[rows, :],
                                         lhsT=wt_t[:, c * P:(c + 1) * P],
                                         rhs=pg_t[:, cc * 512:(cc + 1) * 512],
                                         start=(c == 0), stop=(c == 3),
                                         tile_position=(0, 64 * e))
                    osl = o_t[rows, pp * 512:(pp + 1) * 512]
                    if e == 0:
                        nc.vector.tensor_scalar_add(out=osl, in0=pO[rows, :],
                                                    scalar1=bias_t[rows, :])
                    else:
                        nc.scalar.add(out=osl, in_=pO[rows, :],
                                      add=bias_t[rows, :])
            qeng = (nc.scalar, nc.sync)[sg % 2]
            qeng.dma_start(out=out[sg], in_=o_t[:])

    nc.compile()
    return nc


def _host_indices(src_indices, tgt_perm, relationships):
    """q_sub, q_obj: [L, B, R] int64 — matched query slot per relation."""
    src = np.asarray(src_indices, dtype=np.int64)
    tgt = np.asarray(tgt_perm, dtype=np.int64)
    rel = np.asarray(relationships, dtype=np.int64)

    # lookup[l, b, tgt[l, b, k]] = k
    lookup = np.empty((L, B, M), dtype=np.int64)
    li = np.arange(L)[:, None, None]
    bi = np.arange(B)[None, :, None]
    lookup[li, bi, tgt] = np.broadcast_to(np.arange(M), (L, B, M))

    sub_t = np.broadcast_to(rel[None, :, :, 0], (L, B, R))
    obj_t = np.broadcast_to(rel[None, :, :, 1], (L, B, R))
    pos_sub = np.take_along_axis(lookup, sub_t, axis=2)
    pos_obj = np.take_along_axis(lookup, obj_t, axis=2)
    q_sub = np.take_along_axis(src, pos_sub, axis=2)
    q_obj = np.take_along_axis(src, pos_obj, axis=2)
    return q_sub, q_obj


def _host_prepare(hs, src_indices, tgt_perm, relationships, W_pred, b_pred):
    """Build per-core input maps."""
    import ml_dtypes
    bf16 = ml_dtypes.bfloat16

    hs_bf = np.asarray(hs, dtype=np.float32).astype(bf16)
    W = np.asarray(W_pred, dtype=np.float32)
    b = np.asarray(b_pred, dtype=np.float32)

    q_sub, q_obj = _host_indices(src_indices, tgt_perm, relationships)
    q_cat = np.concatenate([q_sub, q_obj], axis=-1)          # [L, B, 2R]
    # gathered[l, b, j, :] = hs[l, b, q_cat[l, b, j], :]
    gathered = np.take_along_axis(hs_bf, q_cat[..., None], axis=2)

    # W chunks: wt[:, c*P + p] = W[p, c*128 + dd]
    wt_packed = np.ascontiguousarray(
        W.reshape(P, 4, 128).transpose(2, 1, 0).reshape(128, 4 * P)
    ).astype(bf16)
    # bias at partitions 0:51 (even groups) and 64:115 (odd groups)
    bias_col = np.zeros((128, 1), dtype=np.float32)
    bias_col[0:P, 0] = b
    bias_col[64:64 + P, 0] = b

    in_maps = []
    for core in range(NCORES):
        sl = slice(core * BLOC, (core + 1) * BLOC)
        # cols per super-group must be (gs, c=(so, dh), j, r), d on partitions
        arr = gathered[:, sl].reshape(NB, 2, R, 2, 128)   # [nb, so, r, dh, dd]
        arr = arr.reshape(NSG, GD, G, 2, R, 2, 128)       # [sg,gs,j,so,r,dh,dd]
        pg_core = np.ascontiguousarray(
            arr.transpose(0, 6, 1, 3, 5, 2, 4))           # [sg,dd,gs,so,dh,j,r]
        in_maps.append({
            "pg": pg_core.reshape(NSG, 128, GD * 4 * 512),
            "wt": wt_packed,
            "bias": bias_col,
        })
    return in_maps


def kernel(hs, src_indices, tgt_perm, relationships, W_pred, b_pred):
    if "concourse" not in sys.modules:
        try:
            import concourse  # noqa: F401
        except ImportError:
            sys.path.insert(0, "/opt/trn_rl_repo")
    from concourse import bass_utils

    in_maps = _host_prepare(hs, src_indices, tgt_perm, relationships,
                            W_pred, b_pred)
    if "nc" not in _CACHE:
        _CACHE["nc"] = _build_program()
    nc = _CACHE["nc"]

    res = bass_utils.run_bass_kernel_spmd(nc, in_maps, list(range(NCORES)))
    outs = []
    for core in range(NCORES):
        o = res.results[core]["out"]              # [NSG, 128, (GD//2)*512]
        o = np.asarray(o, dtype=np.float32)
        t = o.reshape(NSG, 128, GD // 2, G, R)    # [sg, row, pp, j, r]
        comb = np.stack([t[:, 0:P], t[:, 64:64 + P]], axis=2)
        # comb: [sg, p, e, pp, j, r] -> [sg, pp, e, j, r, p]
        arr = comb.transpose(0, 3, 2, 4, 5, 1).reshape(NB, R, P)
        outs.append(arr.reshape(L, BLOC, R, P))
    return np.concatenate(outs, axis=1)


# revision 6
# speedup vs baseline: 1.6979x; 1.0507x over previous
"""DETR scene-graph predicate head on 8 Trainium2 NeuronCores.

Math: logits[l,b,r,:] = concat(hs[l,b,q_sub], hs[l,b,q_obj]) @ W_pred.T + b_pred
where q_sub/q_obj are derived from (tgt_perm inverse, relationships,
src_indices) — pure integer index math, done on host.

Strategy (batch axis sharded 8 ways; L*B/8 = 192 (layer,image) blocks/core):
  - The host performs the gather itself (it owns the indices anyway) and
    ships the gathered pair representations pre-transposed: per block, four
    [128, 64] bf16 chunks c=(sub/obj, d-half) with d on partitions and
    relation r on columns. This removes the on-chip one-hot gather matmuls,
    the psum->bf16 cast traffic, and halves input DMA bytes vs shipping
    hs + one-hot selectors.
  - Kernel per group of G=8 blocks: 4 accumulating matmuls (lhsT = W chunk
    [128, 51] stationary, rhs streams 512 cols = 8 blocks x 64 relations)
    into a [51, 512] psum region. Groups are paired via tile_position
    (0,0)/(0,64): outputs land on psum partitions 0:51 / 64:115 of one
    bank, so the [128, 512] bf16 stores fan across all DMA engines.
  - Input DMA on the gpsimd (SWDGE) queue with staggered load sizes
    [2,2,4,4,4,4,2,2] groups: small loads at the ends shrink pipeline fill
    and drain; 8-16KB/partition descriptors keep ~370 GB/s sustained.
  - A short dense-matmul preamble plus keep-warm matmuls per load hold the
    PE clock (HAM) at 2.4 GHz through DMA-paced stretches.

hs and W_pred are bf16 on-chip (gather is exact; psum accumulates f32) and
the output travels bf16 over DMA before the host casts to f32, giving
~3e-3 relative error vs the f32 reference.
"""

import sys

import numpy as np

L, B, Q1, D = 6, 256, 101, 256
M, R, P = 64, 64, 51
NCORES = 8
BLOC = B // NCORES          # images per core
NB = L * BLOC               # (layer, image) blocks per core
G = 8                       # blocks per group (one 512-col psum region)
NG = NB // G                # groups per core
GCOLS = 4 * 512             # tile cols per group (4 chunks x 512)
SPLITS = [2, 2, 4, 4, 4, 4, 2, 2]   # groups per DMA load
NPAIR = NG // 2

_CACHE = {}


def _build_program():
    import concourse.bacc as bacc
    import concourse.mybir as mybir
    import concourse.tile as tile
    from contextlib import ExitStack

    f32 = mybir.dt.float32
    bf16 = mybir.dt.bfloat16
    nc = bacc.Bacc("TRN2", target_bir_lowering=False, debug=False)

    pg = nc.dram_tensor("pg", [128, NG * GCOLS], bf16,
                        kind="ExternalInput").ap()
    wt = nc.dram_tensor("wt", [128, 4 * P], bf16, kind="ExternalInput").ap()
    bias = nc.dram_tensor("bias", [128, 1], f32, kind="ExternalInput").ap()
    out = nc.dram_tensor("out", [NPAIR, 128, 512], bf16,
                         kind="ExternalOutput").ap()

    with tile.TileContext(nc) as tc, ExitStack() as ctx:
        const = ctx.enter_context(tc.tile_pool(name="const", bufs=1))
        inp = ctx.enter_context(tc.tile_pool(name="inp", bufs=3))
        outp = ctx.enter_context(tc.tile_pool(name="outp", bufs=4))
        psW = ctx.enter_context(tc.tile_pool(name="psW", bufs=1, space="PSUM"))
        psO = ctx.enter_context(tc.tile_pool(name="psO", bufs=6, space="PSUM"))

        wt_t = const.tile([128, 4 * P], bf16)
        nc.sync.dma_start(out=wt_t[:], in_=wt[:])
        bias_t = const.tile([128, 1], f32)
        nc.sync.dma_start(out=bias_t[:], in_=bias[:])

        # HAM warm-up: dense N=512 matmuls push the PE clock 1.2 -> 2.4 GHz
        wu = const.tile([128, 512], bf16)
        nc.vector.memset(wu[:], 0.0)
        wps = psW.tile([128, 512], f32, tag="wu")
        for _ in range(12):
            nc.tensor.matmul(out=wps[:], lhsT=wu[:, 0:128], rhs=wu[:],
                             start=True, stop=True)

        goff = 0
        pair_idx = 0
        for sz in SPLITS:
            pg_t = inp.tile([128, 4 * GCOLS], bf16, tag="pg")
            nc.gpsimd.dma_start(out=pg_t[:, 0:sz * GCOLS],
                                in_=pg[:, goff * GCOLS:(goff + sz) * GCOLS])
            # keep-warm matmuls run while the PE waits on this load
            for _ in range(sz // 2):
                nc.tensor.matmul(out=wps[:], lhsT=wu[:, 0:128], rhs=wu[:],
                                 start=True, stop=True)
            for pp in range(sz // 2):
                pO = psO.tile([128, 512], f32, tag="pO")
                o_t = outp.tile([128, 512], bf16, tag="o")
                for e in range(2):
                    rows = slice(64 * e, 64 * e + P)
                    for c in range(4):
                        cc = (2 * pp + e) * 4 + c
                        nc.tensor.matmul(out=pO[rows, :],
                                         lhsT=wt_t[:, c * P:(c + 1) * P],
                                         rhs=pg_t[:, cc * 512:(cc + 1) * 512],
                                         start=(c == 0), stop=(c == 3),
                                         tile_position=(0, 64 * e))
                    osl = o_t[rows, :]
                    if e == 0:
                        nc.vector.tensor_scalar_add(out=osl, in0=pO[rows, :],
                                                    scalar1=bias_t[rows, :])
                    else:
                        nc.scalar.add(out=osl, in_=pO[rows, :],
                                      add=bias_t[rows, :])
                qeng = (nc.scalar, nc.sync)[pair_idx % 2]
                qeng.dma_start(out=out[pair_idx], in_=o_t[:])
                pair_idx += 1
            goff += sz

    nc.compile()
    return nc


def _host_indices(src_indices, tgt_perm, relationships):
    """q_sub, q_obj: [L, B, R] int64 — matched query slot per relation."""
    src = np.asarray(src_indices, dtype=np.int64)
    tgt = np.asarray(tgt_perm, dtype=np.int64)
    rel = np.asarray(relationships, dtype=np.int64)

    # lookup[l, b, tgt[l, b, k]] = k
    lookup = np.empty((L, B, M), dtype=np.int64)
    li = np.arange(L)[:, None, None]
    bi = np.arange(B)[None, :, None]
    lookup[li, bi, tgt] = np.broadcast_to(np.arange(M), (L, B, M))

    sub_t = np.broadcast_to(rel[None, :, :, 0], (L, B, R))
    obj_t = np.broadcast_to(rel[None, :, :, 1], (L, B, R))
    pos_sub = np.take_along_axis(lookup, sub_t, axis=2)
    pos_obj = np.take_along_axis(lookup, obj_t, axis=2)
    q_sub = np.take_along_axis(src, pos_sub, axis=2)
    q_obj = np.take_along_axis(src, pos_obj, axis=2)
    return q_sub, q_obj


def _host_prepare(hs, src_indices, tgt_perm, relationships, W_pred, b_pred):
    """Build per-core input maps."""
    import ml_dtypes
    bf16 = ml_dtypes.bfloat16

    hs_bf = np.asarray(hs, dtype=np.float32).astype(bf16)
    W = np.asarray(W_pred, dtype=np.float32)
    b = np.asarray(b_pred, dtype=np.float32)

    q_sub, q_obj = _host_indices(src_indices, tgt_perm, relationships)
    q_cat = np.concatenate([q_sub, q_obj], axis=-1)          # [L, B, 2R]
    # gathered[l, b, j, :] = hs[l, b, q_cat[l, b, j], :]
    gathered = np.take_along_axis(hs_bf, q_cat[..., None], axis=2)

    # W chunks: wt[:, c*P + p] = W[p, c*128 + dd]
    wt_packed = np.ascontiguousarray(
        W.reshape(P, 4, 128).transpose(2, 1, 0).reshape(128, 4 * P)
    ).astype(bf16)
    # bias at partitions 0:51 (even groups) and 64:115 (odd groups)
    bias_col = np.zeros((128, 1), dtype=np.float32)
    bias_col[0:P, 0] = b
    bias_col[64:64 + P, 0] = b

    in_maps = []
    for core in range(NCORES):
        sl = slice(core * BLOC, (core + 1) * BLOC)
        # cols must be (group, c=(so, dh), j, r), d on partitions
        arr = gathered[:, sl].reshape(NB, 2, R, 2, 128)   # [nb, so, r, dh, dd]
        arr = arr.reshape(NG, G, 2, R, 2, 128)            # [g, j, so, r, dh,dd]
        pg_core = np.ascontiguousarray(
            arr.transpose(5, 0, 2, 4, 1, 3))              # [dd, g, so, dh, j,r]
        in_maps.append({
            "pg": pg_core.reshape(128, NG * GCOLS),
            "wt": wt_packed,
            "bias": bias_col,
        })
    return in_maps


def kernel(hs, src_indices, tgt_perm, relationships, W_pred, b_pred):
    if "concourse" not in sys.modules:
        try:
            import concourse  # noqa: F401
        except ImportError:
            sys.path.insert(0, "/opt/trn_rl_repo")
    from concourse import bass_utils

    in_maps = _host_prepare(hs, src_indices, tgt_perm, relationships,
                            W_pred, b_pred)
    if "nc" not in _CACHE:
        _CACHE["nc"] = _build_program()
    nc = _CACHE["nc"]

    res = bass_utils.run_bass_kernel_spmd(nc, in_maps, list(range(NCORES)))
    outs = []
    for core in range(NCORES):
        o = res.results[core]["out"]              # [NPAIR, 128, 512] bf16
        o = np.asarray(o, dtype=np.float32)
        t = o.reshape(NPAIR, 128, G, R)           # [pairi, row, j, r]
        comb = np.stack([t[:, 0:P], t[:, 64:64 + P]], axis=1)
        # comb: [pairi, e, p, j, r] -> [pairi, e, j, r, p]
        arr = comb.transpose(0, 1, 3, 4, 2).reshape(NB, R, P)
        outs.append(arr.reshape(L, BLOC, R, P))
    return np.concatenate(outs, axis=1)
